# revision 19
# baseline (speedup 1.0000x reference)
"""Trainium2 Bass kernel for nn_CrossAttention (B=8, C=256, H=W=64, inter=32).

Math (per batch sample b):
    Q = Wq @ xg + bq          (32, 4096)   xg = gaf_features[b]  (256, 4096)
    K = Wk @ xm + bk          (32, 4096)   xm = mtf_features[b]
    V = Wv @ xm + bv          (32, 4096)
    L[k, q]   = sum_c K[c, k] Q[c, q]          (4096, 4096)
    A[k, q]   = exp(L[k, q]) / sum_q' exp(L[k, q'])     (softmax over q)
    out[c, q] = sum_k V[c, k] A[k, q]          (32, 4096)
    res       = gamma * (Wo @ out + bo) + xg   (256, 4096)

Sharding: data-parallel over batch — core i handles sample i (8 cores, B=8).

Per-core kernel structure (v2 — ScalarE-bound pipeline, ~saturated exp):
  - k-dim processed in 32 tiles of 128 (k on PSUM/SBUF partitions).
  - L per k-tile staged in PSUM as 4 chunks of 1024 through 3 rotating
    2-bank slots; 3-deep rotation keeps the exp ACTIVATEs gapless (each
    chunk's matmuls overlap the exp of a chunk two behind).
  - exp has NO accum_out (the ACTIVATION_READ_ACCUMULATOR drain costs
    ~287ns/chunk); instead Z row-sums are computed off the bf16 P tile:
    DVE reduces q[0:2048], GPSIMD reduces q[2048:4096] (both otherwise
    idle), combined + reciprocal + folded into V^T rows on DVE.
  - AV accumulated over all 32 k-tiles in PSUM (2 banks) via col-tiled
    matmuls (tile_position=(0,32j)); dummy zero matmul initializes.
  - K chunks and V^T tiles are produced just-in-time inside the k-tile
    loop (they only gate later k-tiles), so the prologue is just:
    weight DMAs -> input DMAs (interleaved xg/xm order) -> Q chunks ->
    K0 -> vt0..3, and the first exp fires within a few microseconds.
  - residual base xgb = xg + gamma*bo precomputed on DVE during the
    main loop; epilogue is matmul -> single fused DVE op -> DMA per
    512-column chunk, all pipelined.
  - softmax max-subtraction is skipped: logits are bounded (|L| < ~10)
    by construction, exp is exact fp32.
"""

import numpy as np

C = 256          # in channels
D = 32           # inter channels
HW = 4096        # H*W
P = 128
NKT = HW // P    # 32 k-tiles
NQC = HW // 512  # 8 q-chunks of 512
B = 8
H = W = 64

_CACHE = {}
PROFILE = False           # set True (e.g. from test.py) to collect a trace
LAST_EXEC_NS = None
LAST_RESULTS = None
Z_MODE = "tree"           # "tree" (pairwise bf16 adds at 2x) or "reduce"


def _build_nc():
    import concourse.tile as tile
    from concourse import bacc, mybir

    F32 = mybir.dt.float32
    BF16 = mybir.dt.bfloat16
    Act = mybir.ActivationFunctionType
    Alu = mybir.AluOpType
    AxX = mybir.AxisListType.X

    nc = bacc.Bacc()

    xg_h = nc.declare_dram_parameter("xg", [C, HW], F32, isOutput=False)
    xm_h = nc.declare_dram_parameter("xm", [C, HW], F32, isOutput=False)
    # wqkv: [C, 288] = WqT-rep4 | WkT-rep4 | WvT packed (single DMA)
    wqkv_h = nc.declare_dram_parameter("wqkv", [C, 2 * P + D], F32, isOutput=False)
    wo_h = nc.declare_dram_parameter("wo", [D, C], F32, isOutput=False)  # Wo^T
    # consts [P, 37]: bvb(0:32), bo2(32:34), gmb(34:35), bq(35) rows 0:32,
    # bk(36) rows 0:32
    cst_h = nc.declare_dram_parameter("cst", [P, 37], F32, isOutput=False)
    res_h = nc.declare_dram_parameter("res", [C, HW], F32, isOutput=True)

    xg = xg_h[:].rearrange("(o p) q -> p o q", p=P)   # c = o*128 + p
    xm = xm_h[:].rearrange("(o p) q -> p o q", p=P)
    res = res_h[:].rearrange("(o p) q -> p o q", p=P)
    wqkv = wqkv_h[:].rearrange("(o p) d -> p o d", p=P)

    with tile.TileContext(nc) as tc:
        with (
            tc.tile_pool(name="singles", bufs=1) as singles,
            tc.tile_pool(name="ppool", bufs=4) as ppool,
            tc.tile_pool(name="ztpool", bufs=2) as ztpool,
            tc.tile_pool(name="lpool", bufs=3, space="PSUM") as lpool,
            tc.tile_pool(name="opool", bufs=1, space="PSUM") as opool,
            tc.tile_pool(name="small", bufs=6) as small,
            tc.tile_pool(name="respool", bufs=4) as respool,
        ):
            # ---------------- constants ----------------
            # wqkv layout: [wq4 (128) | wk4 (128) | wv (32)] where wq4/wk4
            # have Wq^T/Wk^T replicated 4x along stationary columns, so the
            # Q/K projections write all 4 partition strips directly (enables
            # row-tiled L matmuls with zero extra cost).
            wqkv_s = singles.tile([P, 2, 2 * P + D], F32, name="wqkv_s")
            nc.sync.dma_start(out=wqkv_s, in_=wqkv)
            wq_s = wqkv_s[:, :, 0:P]
            wk_s = wqkv_s[:, :, P : 2 * P]
            wv_s = wqkv_s[:, :, 2 * P : 2 * P + D]
            wo_s = singles.tile([P, C], F32, name="wo_s")
            for j in range(4):  # replicate Wo^T into the 4 partition strips
                nc.sync.dma_start(out=wo_s[32 * j : 32 * (j + 1), :], in_=wo_h[:])
            cst_s = singles.tile([P, 37], F32, name="cst_s")
            nc.sync.dma_start(out=cst_s, in_=cst_h[:])
            bvb_s = cst_s[:, 0:D]
            bo_s = cst_s[:, D : D + 2]
            gm_s = cst_s[:, D + 2 : D + 3]
            bq_s = cst_s[:, D + 3 : D + 4]   # bq tiled 4x over partitions
            bk_s = cst_s[:, D + 4 : D + 5]   # bk tiled 4x over partitions
            gbo_s = singles.tile([P, 2], F32, name="gbo_s")
            nc.vector.tensor_scalar_mul(gbo_s, bo_s, gm_s)  # gamma * bo
            zero_s = singles.tile([P, 512], F32, name="zero_s")
            nc.vector.memset(zero_s, 0.0)
            # warm the exp table (ACT_TABLE_LOAD ~2.7us) during the prologue
            warm_s = small.tile([P, 1], F32, name="warm_s")
            nc.scalar.activation(out=warm_s, in_=cst_s[:, 0:1], func=Act.Exp)

            # input feature tiles (kept resident; xg also used for residual)
            xg_s = singles.tile([P, 2, HW], F32, name="xg_s")
            xm_s = singles.tile([P, 2, HW], F32, name="xm_s")
            # DMA order: xm0 first (K0/vt0-3), then xg (Q gates the loop
            # start), xm1 early (K1/vt4 are produced at k-tiles 1-3).
            def dma_xg(qc):
                sl = slice(512 * qc, 512 * (qc + 1))
                nc.sync.dma_start(out=xg_s[:, :, sl], in_=xg[:, :, sl])

            def dma_xm(mc):
                sl = slice(512 * mc, 512 * (mc + 1))
                nc.sync.dma_start(out=xm_s[:, :, sl], in_=xm[:, :, sl])

            dma_xg(0)
            dma_xg(1)
            dma_xm(0)
            for qc in range(2, 8):
                dma_xg(qc)
            for mc in range(1, 8):
                dma_xm(mc)

            # Q/K chunk tiles, replicated over the 4 partition strips
            q_tiles = [singles.tile([P, 512], BF16, name=f"q_t{i}") for i in range(NQC)]
            k_tiles = [singles.tile([P, 512], BF16, name=f"k_t{i}") for i in range(NQC)]
            vt_tiles = [
                singles.tile([P, D], F32, name=f"vt_t{t}") for t in range(NKT)
            ]
            # residual base: xgb = xg + gamma*bo (computed during main loop)
            xgb_s = singles.tile([P, 2, HW], F32, name="xgb_s")

            # persistent col-packed output accumulator: strip j of bank b holds
            # out[:, 512*(4b+j) : 512*(4b+j)+512]
            out_ps = opool.tile([P, 1024], F32, name="out_ps")
            # dummy zero matmuls: clear has_written for both banks, data = 0
            for b in range(2):
                nc.tensor.matmul(
                    out=out_ps[:, 512 * b : 512 * (b + 1)],
                    lhsT=zero_s[:, :P],
                    rhs=zero_s[:, :512],
                    start=True,
                    stop=False,
                    skip_group_check=True,
                )
            # PE warm-up: ~3.5us of junk matmuls while the input DMAs land,
            # so HAM unthrottles (1.2 -> 2.4 GHz) before the real stream.
            warm_ps = lpool.tile([P, 512], F32, tag="lc", name="warm_ps")
            for _ in range(8):
                nc.tensor.matmul(
                    out=warm_ps, lhsT=zero_s[:, :P], rhs=zero_s[:, :512],
                    start=True, stop=True,
                )

            # ---------------- projection helpers ----------------
            def produce_q(qc):
                sl = slice(512 * qc, 512 * (qc + 1))
                q_ps = lpool.tile([P, 512], F32, tag="lc", name="q_ps")
                nc.tensor.matmul(
                    out=q_ps, lhsT=wq_s[:, 0, :], rhs=xg_s[:, 0, sl],
                    start=True, stop=False,
                )
                nc.tensor.matmul(
                    out=q_ps, lhsT=wq_s[:, 1, :], rhs=xg_s[:, 1, sl],
                    start=False, stop=True,
                )
                nc.vector.tensor_scalar_add(q_tiles[qc], q_ps, bq_s)

            def produce_k(kc):
                sl = slice(512 * kc, 512 * (kc + 1))
                k_ps = lpool.tile([P, 512], F32, tag="lc", name="k_ps")
                nc.tensor.matmul(
                    out=k_ps, lhsT=wk_s[:, 0, :], rhs=xm_s[:, 0, sl],
                    start=True, stop=False,
                )
                nc.tensor.matmul(
                    out=k_ps, lhsT=wk_s[:, 1, :], rhs=xm_s[:, 1, sl],
                    start=False, stop=True,
                )
                nc.vector.tensor_scalar_add(k_tiles[kc], k_ps, bk_s)

            def produce_vt(t):
                # vt[k, c] = sum_ch xm[ch, k] * WvT[ch, c]  (+ bv broadcast)
                ksl = slice(P * t, P * (t + 1))
                vt_ps = lpool.tile([P, D], F32, tag="lc", name="vt_ps")
                nc.tensor.matmul(
                    out=vt_ps, lhsT=xm_s[:, 0, ksl], rhs=wv_s[:, 0, :],
                    start=True, stop=False,
                )
                nc.tensor.matmul(
                    out=vt_ps, lhsT=xm_s[:, 1, ksl], rhs=wv_s[:, 1, :],
                    start=False, stop=True,
                )
                nc.vector.tensor_add(vt_tiles[t], vt_ps, bvb_s)

            def produce_xgb(chunk):
                h, qc = chunk // NQC, chunk % NQC
                sl = slice(512 * qc, 512 * (qc + 1))
                nc.vector.tensor_scalar_add(
                    xgb_s[:, h, sl], xg_s[:, h, sl], gbo_s[:, h : h + 1]
                )

            # ---------------- prologue projections ----------------
            # Emission order tracks DMA arrival order so the strict-FIFO PE
            # queue never head-blocks on a late DMA.
            produce_q(0)
            produce_q(1)
            produce_k(0)
            for t in range(4):
                produce_vt(t)

            def emit_l_chunk(kt, ci):
                kq, ko = kt // 4, (kt % 4) * P
                l_ps = lpool.tile([P, 1024], F32, tag="lc", name="l_ps")
                for j in range(2):
                    qi = 2 * ci + j
                    r = qi % 4          # row group: cycles so LDW pulls ahead
                    nc.tensor.matmul(
                        out=l_ps[:, 512 * j : 512 * (j + 1)],
                        lhsT=k_tiles[kq][32 * r : 32 * (r + 1), ko : ko + P],
                        rhs=q_tiles[qi][32 * r : 32 * (r + 1), :],
                        tile_position=(32 * r, 0),
                        start=True,
                        stop=True,
                    )
                return l_ps

            # ---------------- main loop over k-tiles ----------------
            # Software-pipelined IR: L+exp of k-tile kt is emitted BEFORE
            # Z/AV/trickle of k-tile kt-1, so the static per-engine queue
            # order lets the next k-tile's L matmuls run during the current
            # k-tile's exps (instead of behind the AV/projection matmuls).
            p_tiles = {}

            def stage_a(kt):
                p_t = ppool.tile([P, HW], BF16, tag="p", name="p_t")
                p_tiles[kt] = p_t
                for ci in range(4):
                    if kt == 0 and ci > 0:   # interleave remaining Q chunks
                        produce_q(2 * ci)
                        produce_q(2 * ci + 1)
                    l_ps = emit_l_chunk(kt, ci)
                    nc.scalar.activation(
                        out=p_t[:, 1024 * ci : 1024 * (ci + 1)],
                        in_=l_ps[:, :1024],
                        func=Act.Exp,
                    )

            def stage_b(kt):
                p_t = p_tiles.pop(kt)
                # Z[k] = sum_q P[k, q]: pairwise bf16 adds run at 2x DVE
                # rate, plain reduce only at 1x -- so tree down to 512 wide.
                zs = small.tile([P, 1], F32, name="zs")
                if Z_MODE == "tree":
                    zt2 = ztpool.tile([P, 2048], BF16, tag="zt2", name="zt2")
                    nc.vector.tensor_add(zt2, p_t[:, 0:2048], p_t[:, 2048:HW])
                    zt1 = ztpool.tile([P, 1024], BF16, tag="zt1", name="zt1")
                    nc.vector.tensor_add(zt1, zt2[:, 0:1024], zt2[:, 1024:2048])
                    zt0 = ztpool.tile([P, 512], BF16, tag="zt0", name="zt0")
                    nc.vector.tensor_add(zt0, zt1[:, 0:512], zt1[:, 512:1024])
                    nc.vector.reduce_sum(out=zs, in_=zt0, axis=AxX)
                else:
                    nc.vector.reduce_sum(out=zs, in_=p_t[:, 0:HW], axis=AxX)
                zr = small.tile([P, 1], F32, name="zr")
                nc.vector.reciprocal(out=zr, in_=zs)
                vts = small.tile([P, D], BF16, name="vts")
                nc.vector.tensor_scalar_mul(vts, vt_tiles[kt], zr)

                for b in range(2):
                    for j in range(4):
                        qi = 4 * b + j
                        nc.tensor.matmul(
                            out=out_ps[32 * j : 32 * (j + 1), 512 * b : 512 * (b + 1)],
                            lhsT=vts,
                            rhs=p_t[:, 512 * qi : 512 * (qi + 1)],
                            tile_position=(0, 32 * j),
                            start=False,
                            stop=(kt == NKT - 1 and j == 3),
                            skip_group_check=True,
                        )

                # just-in-time production for later k-tiles / epilogue
                if kt % 4 == 1 and kt < 28:
                    produce_k(kt // 4 + 1)
                if 2 <= kt <= 29:
                    produce_vt(kt + 2)
                if kt < 16:
                    produce_xgb(kt)

            for kt in range(NKT + 1):
                if kt < NKT:
                    stage_a(kt)
                if kt >= 1:
                    stage_b(kt - 1)

            # ---------------- epilogue: Wo projection + residual ----------------
            out4_s = singles.tile([P, 1024], F32, name="out4_s")
            nc.vector.tensor_copy(out=out4_s, in_=out_ps)
            for h in range(2):          # co half
                for part in range(2):   # q-chunk within strip
                    for j in range(4):  # strip (row group)
                        qi = 4 * part + j
                        qsl = slice(512 * qi, 512 * (qi + 1))
                        o2_ps = lpool.tile([P, 512], F32, tag="lc", name="o2_ps")
                        nc.tensor.matmul(
                            out=o2_ps,
                            lhsT=wo_s[32 * j : 32 * (j + 1), P * h : P * (h + 1)],
                            rhs=out4_s[32 * j : 32 * (j + 1),
                                       512 * part : 512 * (part + 1)],
                            tile_position=(32 * j, 0),
                            start=True,
                            stop=True,
                        )
                        res_s = respool.tile([P, 512], F32, name="res_s")
                        # res = gamma*o2 + (xg + gamma*bo)
                        nc.vector.scalar_tensor_tensor(
                            out=res_s,
                            in0=o2_ps,
                            scalar=gm_s,
                            op0=Alu.mult,
                            in1=xgb_s[:, h, qsl],
                            op1=Alu.add,
                        )
                        nc.sync.dma_start(out=res[:, h, qsl], in_=res_s)

    nc.finalize()
    return nc


def _get_nc():
    if "nc" not in _CACHE:
        _CACHE["nc"] = _build_nc()
    return _CACHE["nc"]


def _make_in_maps(gaf, mtf, Wq, bq, Wk, bk, Wv, bv, Wo, bo, gamma):
    f = np.float32
    # Wq^T / Wk^T replicated 4x along stationary columns (row-tiled L mms)
    wqkv = np.concatenate(
        [np.tile(Wq.T, (1, 4)), np.tile(Wk.T, (1, 4)), Wv.T], axis=1
    ).astype(f)                                                   # (256, 288)
    wo = np.ascontiguousarray(Wo.T, dtype=f)                      # (32, 256)
    cst = np.zeros((P, 37), f)
    cst[:, 0:D] = np.broadcast_to(bv.reshape(1, D), (P, D))       # bvb
    cst[:, D:D + 2] = bo.reshape(2, P).T                          # bo2 [p, o]
    cst[:, D + 2] = np.asarray(gamma).reshape(-1)[0]              # gamma bcast
    cst[:, D + 3] = np.tile(bq, 4)                                # bq rep4
    cst[:, D + 4] = np.tile(bk, 4)                                # bk rep4
    shared = dict(wqkv=np.ascontiguousarray(wqkv), wo=wo, cst=cst)
    in_maps = []
    for b in range(B):
        m = dict(shared)
        m["xg"] = np.ascontiguousarray(gaf[b].reshape(C, HW), dtype=f)
        m["xm"] = np.ascontiguousarray(mtf[b].reshape(C, HW), dtype=f)
        in_maps.append(m)
    return in_maps


def kernel(gaf_features, mtf_features, Wq, bq, Wk, bk, Wv, bv, Wo, bo, gamma):
    global LAST_EXEC_NS, LAST_RESULTS
    from concourse.bass_utils import run_bass_kernel_spmd

    nc = _get_nc()
    in_maps = _make_in_maps(
        np.asarray(gaf_features), np.asarray(mtf_features),
        np.asarray(Wq), np.asarray(bq), np.asarray(Wk), np.asarray(bk),
        np.asarray(Wv), np.asarray(bv), np.asarray(Wo), np.asarray(bo),
        np.asarray(gamma),
    )
    core_ids = list(range(B))
    r = run_bass_kernel_spmd(nc, in_maps, core_ids, trace=PROFILE)
    LAST_EXEC_NS = r.exec_time_ns
    LAST_RESULTS = r
    out = np.stack([r.results[i]["res"] for i in range(B)], axis=0)
    return out.reshape(B, C, H, W).astype(np.float32)
